# revision 15
# baseline (speedup 1.0000x reference)
"""Trainium2 Bass kernel for nn_Conv2dMap (grouped-gather 3x3 conv).

Problem: x [32, 8, 512, 512] f32, weight [16, 4, 3, 3], bias [16].
Output channel i convolves input channels {(i-3)%8..i%8} (4 of them) with a
3x3 kernel, VALID padding -> out [32, 16, 510, 510].

Strategy
--------
Data parallel: batch 32 sharded 4-per-core across 8 NeuronCores, weights
replicated. No collectives.

Per core, the conv is lowered to matmuls:
  - Scatter weight into dense w_dense [16, 8, 3, 3].
  - Output rows are processed in blocks of DY=8. For one block starting at
    output row r0 we load input rows r0..r0+9 (yy' in 0..9) of all 8 input
    channels into SBUF as rhs [(yy', ic)=80 partitions, 512 cols].
  - lhsT W3[kx] [(yy', ic)=80, (oc, dy)=128]:
        W3[kx][yy*8+ic, oc*8+dy] = w_dense[oc, ic, yy-dy, kx]  (0 <= yy-dy < 3)
  - 3 matmuls (kx = 0..2) accumulate into PSUM [128, 510], reading rhs with
    the free dim shifted by kx:  psum[(oc,dy), x] = conv(out row r0+dy, col x).
  - ScalarE activation(Copy, bias) adds the per-channel bias, result DMAs to
    out[n, oc, r0+dy, :].
Row blocks: 64 per image; the last block starts at row 502 (overlaps block 62
by 2 rows -> benign identical double-write).
"""

import os
import sys

import numpy as np

_TRN_REPO = "/opt/trn_rl_repo"
if _TRN_REPO not in sys.path:
    sys.path.insert(0, _TRN_REPO)

IN_C, OUT_C, K_PER, KS = 8, 16, 4, 3
N_CORES = 8
DY = 8                 # output rows per block
YYP = DY + KS - 1      # input rows per block (10)
KP = YYP * IN_C        # rhs partitions (80)
M = OUT_C * DY         # psum partitions (128)

# dtype used for the matmul operands: "float32" (exact, 4 cyc/row) or
# "float32r" (full-rate fp32 path, TF32-like) — flipped during optimization.
MM_DTYPE = os.environ.get("CONV_MM_DTYPE", "float32")

_cache = {}


def _mapping_idx():
    mapping = np.array(
        [[1 if (c - i) % IN_C < K_PER else 0 for i in range(OUT_C)]
         for c in range(IN_C)], dtype=np.int64)
    return np.stack([np.nonzero(mapping[:, i])[0] for i in range(OUT_C)])


def _build_weights(weight, bias):
    """Host-side prep of lhsT stacks + bias vector (tiny)."""
    idx = _mapping_idx()
    w_dense = np.zeros((OUT_C, IN_C, KS, KS), np.float32)
    w_dense[np.arange(OUT_C)[:, None], idx] = weight.astype(np.float32)
    W3 = np.zeros((KS, KP, M), np.float32)
    for ky in range(KS):
        for dy in range(DY):
            yy = dy + ky
            # W3[kx, yy*8+ic, oc*8+dy] = w_dense[oc, ic, ky, kx]
            W3[:, yy * IN_C:(yy + 1) * IN_C, dy::DY] = (
                w_dense[:, :, ky, :].transpose(2, 1, 0))
    b128 = np.repeat(bias.astype(np.float32), DY)[:, None].copy()
    return W3, b128


def _build_bass(nb, h, w, mm_dtype, split_waits=True):
    """Build the per-core Bass program. nb images of [IN_C, h, w] -> nb
    outputs of [OUT_C, h-2, w-2]."""
    import concourse.bass as bass
    import concourse.mybir as mybir
    import concourse.tile as tile

    f32 = mybir.dt.float32
    mmdt = getattr(mybir.dt, mm_dtype)
    oh, ow = h - 2, w - 2
    n_blocks = (oh + DY - 1) // DY

    nc = bass.Bass(trn_type="TRN2")
    x = nc.declare_dram_parameter("x", [nb, IN_C, h, w], f32, isOutput=False)
    w3 = nc.declare_dram_parameter("w3", [KS, KP, M], f32, isOutput=False)
    b128 = nc.declare_dram_parameter("b128", [M, 1], f32, isOutput=False)
    out = nc.declare_dram_parameter("out", [nb, OUT_C, oh, ow], f32, isOutput=True)

    with tile.TileContext(nc) as tc:
        with (
            tc.tile_pool(name="const", bufs=1) as cpool,
            tc.tile_pool(name="rhs", bufs=4) as rpool,
            tc.tile_pool(name="res", bufs=4) as opool,
            tc.tile_pool(name="psum", bufs=4, space="PSUM") as ppool,
        ):
            w_sb = cpool.tile([KP, KS, M], f32)
            nc.sync.dma_start(out=w_sb, in_=w3.ap().transpose([1, 0, 2]))
            b_sb = cpool.tile([M, 1], f32)
            nc.sync.dma_start(out=b_sb, in_=b128.ap())

            for n in range(nb):
                for b in range(n_blocks):
                    r0 = min(b * DY, oh - DY)
                    rhs = rpool.tile([KP, w], f32)
                    # SBUF side must be 2D (partition dim = dim0 only); the
                    # DRAM side carries the (row, chan, col) structure. DMA
                    # pairs elements in flattened order: p = yy*8 + ic.
                    nc.sync.dma_start(
                        out=rhs[:, :],
                        in_=x[n, :, r0:r0 + YYP, :].transpose([1, 0, 2]),
                    )
                    ps = ppool.tile([M, ow], f32)
                    for kx in range(KS):
                        nc.tensor.matmul(
                            ps,
                            lhsT=w_sb[:, kx, :].bitcast(mmdt),
                            rhs=rhs[:, kx:kx + ow].bitcast(mmdt),
                            start=(kx == 0),
                            stop=(kx == KS - 1),
                        )
                    res = opool.tile([M, ow], f32)
                    nc.scalar.activation(
                        res, ps, mybir.ActivationFunctionType.Identity, bias=b_sb)
                    nc.sync.dma_start(
                        out=out[n, :, r0:r0 + DY, :],
                        in_=res[:, :],
                    )
    if split_waits:
        # CoreSim's race detector rejects post-hoc instructions, so this
        # runs only for HW builds.
        _split_excess_waits(nc)
    return nc


def _split_excess_waits(nc, max_keep=1):
    """This walrus build allows only ONE semaphore wait per instruction
    (setupSyncWait: 'Too many sync wait commands'). Move excess waits onto
    standalone InstEventSemaphore instructions placed immediately before the
    over-constrained instruction on the same engine queue — the sequencer
    executes them in order, so semantics are identical."""
    import concourse.mybir as mybir

    for f in nc.m.functions:
        for bb in f.blocks:
            out, changed = [], False
            for inst in bb.instructions:
                si = inst.sync_info
                if (si is not None and si.on_wait
                        and len(si.on_wait) > max_keep
                        and not isinstance(inst, mybir.InstEventSemaphore)):
                    waits = list(si.on_wait)
                    excess, keep = waits[:-max_keep], waits[-max_keep:]
                    for j, wt in enumerate(excess):
                        out.append(mybir.InstEventSemaphore(
                            name=f"{inst.name}-w{j}",
                            engine=inst.engine,
                            sync_info=mybir.SyncInfo(on_wait=[wt], on_update=[]),
                            bass_nofuse=True,
                        ))
                    inst.sync_info = mybir.SyncInfo(
                        on_wait=keep, on_update=list(si.on_update or []))
                    changed = True
                out.append(inst)
            if changed:
                bb.instructions = out


def get_nc(nb=4, h=512, w=512, mm_dtype=MM_DTYPE, split_waits=True):
    key = (nb, h, w, mm_dtype, split_waits)
    if key not in _cache:
        _cache[key] = _build_bass(nb, h, w, mm_dtype, split_waits)
    return _cache[key]


class _Runner:
    """Compiled SPMD runner for a Bass program, cached across calls.

    Mirrors bass2jax.run_bass_via_pjrt but keeps the jitted executable so
    repeat invocations don't recompile the NEFF.
    """

    def __init__(self, nc):
        import jax
        import numpy as _np
        from jax.sharding import Mesh, PartitionSpec
        from jax.experimental.shard_map import shard_map
        import concourse.mybir as mybir
        from concourse import bass2jax

        bass2jax.install_neuronx_cc_hook()
        assert nc.dbg_addr is None or not nc.dbg_callbacks
        self.nc = nc
        partition_name = (nc.partition_id_tensor.name
                          if nc.partition_id_tensor else None)

        in_names, out_names, out_avals, zero_shapes = [], [], [], []
        for alloc in nc.m.functions[0].allocations:
            if not isinstance(alloc, mybir.MemoryLocationSet):
                continue
            name = alloc.memorylocations[0].name
            if alloc.kind == "ExternalInput":
                if name != partition_name:
                    in_names.append(name)
            elif alloc.kind == "ExternalOutput":
                shape = tuple(alloc.tensor_shape)
                dtype = mybir.dt.np(alloc.dtype)
                out_names.append(name)
                out_avals.append(jax.core.ShapedArray(shape, dtype))
                zero_shapes.append((shape, dtype))
        self.in_names = list(in_names)
        self.out_names = out_names
        self.out_avals = out_avals
        self.zero_shapes = zero_shapes
        n_params = len(in_names)
        n_outs = len(out_names)
        all_names = in_names + out_names
        if partition_name is not None:
            all_names.append(partition_name)
        donate = tuple(range(n_params, n_params + n_outs))
        self.n_params = n_params

        from concourse.bass2jax import _bass_exec_p, partition_id_tensor

        def _body(*args):
            operands = list(args)
            if partition_name is not None:
                operands.append(partition_id_tensor())
            outs = _bass_exec_p.bind(
                *operands,
                out_avals=tuple(out_avals),
                in_names=tuple(all_names),
                out_names=tuple(out_names),
                lowering_input_output_aliases=(),
                sim_require_finite=True,
                sim_require_nnan=True,
                nc=nc,
            )
            return tuple(outs)

        devices = jax.devices()[:N_CORES]
        assert len(devices) == N_CORES
        self.mesh = Mesh(_np.asarray(devices), ("core",))
        in_specs = (PartitionSpec("core"),) * (n_params + n_outs)
        out_specs = (PartitionSpec("core"),) * n_outs
        self.jitted = jax.jit(
            shard_map(_body, mesh=self.mesh, in_specs=in_specs,
                      out_specs=out_specs, check_rep=False),
            donate_argnums=donate,
            keep_unused=True,
        )

    def concat_inputs(self, in_maps):
        return [
            np.concatenate([np.asarray(m[name]) for m in in_maps], axis=0)
            for name in self.in_names
        ]

    def make_zeros(self):
        return [np.zeros((N_CORES * s[0], *s[1:]), d)
                for s, d in self.zero_shapes]

    def __call__(self, in_maps):
        out_arrs = self.jitted(*self.concat_inputs(in_maps),
                               *self.make_zeros())
        n_out0 = [a.shape[0] for a in self.out_avals]
        return [
            {name: np.asarray(out_arrs[i]).reshape(
                N_CORES, *self.out_avals[i].shape)[c]
             for i, name in enumerate(self.out_names)}
            for c in range(N_CORES)
        ]


_runner_cache = {}


def get_runner(nb=4, h=512, w=512, mm_dtype=MM_DTYPE):
    key = (nb, h, w, mm_dtype)
    if key not in _runner_cache:
        _runner_cache[key] = _Runner(get_nc(nb, h, w, mm_dtype))
    return _runner_cache[key]


def _prep_inputs(x, weight, bias):
    x = np.ascontiguousarray(np.asarray(x, np.float32))
    W3, b128 = _build_weights(np.asarray(weight, np.float32),
                              np.asarray(bias, np.float32))
    n = x.shape[0]
    nb = n // N_CORES
    return [
        {"x": x[i * nb:(i + 1) * nb], "w3": W3, "b128": b128}
        for i in range(N_CORES)
    ]


def kernel(x, weight, bias):
    n, _, h, w = x.shape
    runner = get_runner(n // N_CORES, h, w)
    results = runner(_prep_inputs(x, weight, bias))
    return np.concatenate([r["out"] for r in results], axis=0)


def bench(x, weight, bias, iters=10):
    """Time device-resident repeated execution; returns seconds/iter."""
    import time

    import jax
    from jax.sharding import NamedSharding, PartitionSpec

    n, _, h, w = x.shape
    runner = get_runner(n // N_CORES, h, w)
    sh = NamedSharding(runner.mesh, PartitionSpec("core"))
    dev_in = [jax.device_put(a, sh)
              for a in runner.concat_inputs(_prep_inputs(x, weight, bias))]
    dev_zeros = [[jax.device_put(z, sh) for z in runner.make_zeros()]
                 for _ in range(iters + 1)]
    # warmup
    jax.block_until_ready(runner.jitted(*dev_in, *dev_zeros[iters]))
    t0 = time.perf_counter()
    outs = [runner.jitted(*dev_in, *dev_zeros[i]) for i in range(iters)]
    jax.block_until_ready(outs)
    return (time.perf_counter() - t0) / iters
